# revision 25
# baseline (speedup 1.0000x reference)
"""Causal self-attention Trainium2 Bass kernel (v2).

Sharding: 8-way head tensor-parallelism (2 heads per core, full batch) for
QKV projections + attention; the attention output is re-sharded over tokens
with TWO AllToAlls (pairs a<2 fire at ~30% of attention and hide under the
rest; pairs a>=2 fire at the end), then each core output-projects 512 tokens.

Key differences vs v1:
  - softmax normalization happens AFTER the collective: each (b,a) pair ships
    raw PV rows plus the denominator row (65 rows/head) through the AllToAll;
    the receiver batches all 16 denominators into one [16,256]
    reciprocal_approx_fast and broadcasts via a tiny selector matmul.
  - attention inner loop is software-pipelined (PV of chunk i deferred until
    after S/exp of chunk i+1) so the in-order PE queue never waits on ACT.
  - projections use N=1024 moving operands, are interleaved with attention
    emission, and the Wo load is deferred out of the startup critical path.

Per-core layouts:
  xT     [128, 8, 4096] bf16   x^T arranged (d_inner, d_outer, b*t)
  wq/wk/wv [128, 8, 128] bf16  W[:, head-slice] as (d_inner, d_outer, out)
  wo     [128, 8, 1024] bf16   full Wo
  Q^T/K^T in SBUF [128 (2 heads x 64), 4096] bf16
  V in SBUF [128 (j in chunk), 32 (b*jc), 2 (head), 80 (V | ones | pad)]
  S^T = K^T.T @ Q^T tiles [j=128, i<=512], 2 heads packed in PE row-halves
  PV accumulates [65, 2, 512] (65th row = denominator via ones column of V)
"""

import math
import os

import numpy as np

os.environ.setdefault("JAX_COMPILATION_CACHE_DIR", "/tmp/jax_cache")

D_MODEL = 1024
NUM_HEADS = 16
D_K = 64
B = 2
T = 2048
TT = B * T          # 4096 flattened tokens
NCORES = 8
HL = NUM_HEADS // NCORES   # heads per core = 2
DO = D_MODEL // 128        # 8 contraction chunks
NI = T // 512              # 4 query chunks per batch
NJ = T // 128              # 16 key chunks per batch

# pair processing order (b, a) and the AllToAll groups
PAIR_ORDER = [(0, 0), (1, 0), (0, 1), (1, 1), (0, 2), (1, 2), (0, 3), (1, 3)]
GROUPS = [PAIR_ORDER[:4], PAIR_ORDER[4:6], PAIR_ORDER[6:]]
GTOKS = [64 * len(p) for p in GROUPS]          # tokens per core per a2a
YOFF = [sum(GTOKS[:g]) for g in range(len(GROUPS))]

_cache = {}


def _install_ntff_hook():
    """The agent image's antenv lacks axon_hooks; replicate what
    trn_agent_boot would register so trace=True can capture NTFFs."""
    import sys
    import types

    try:
        from antenv import axon_hooks  # noqa: F401
        return True
    except ImportError:
        pass
    try:
        import antenv
        from trn_agent_boot.trn_boot import _ntff_profile_via_ctypes

        mod = types.ModuleType("antenv.axon_hooks")
        holder = [None]
        mod.set_axon_ntff_profile_hook = lambda h: holder.__setitem__(0, h)
        mod.get_axon_ntff_profile_hook = lambda: holder[0]
        sys.modules["antenv.axon_hooks"] = mod
        antenv.axon_hooks = mod
        mod.set_axon_ntff_profile_hook(
            _ntff_profile_via_ctypes("/opt/axon/libaxon_pjrt.so")
        )
        return True
    except Exception:
        return False


def _build_module(mode, blocks=None, n_mtiles=1):
    """Build + compile the Bass module.

    mode: "causal" (tril mask), "ones" (no masking), "generic"
    blocks: for generic mode, blocks[jc][a] = 0 skip / 1 full / (2, idx) mixed
    """
    from contextlib import ExitStack

    import concourse.mybir as mybir
    import concourse.tile as tile
    from concourse import bacc

    F32 = mybir.dt.float32
    BF16 = mybir.dt.bfloat16
    AF = mybir.ActivationFunctionType

    nc = bacc.Bacc(
        "TRN2",
        target_bir_lowering=False,
        debug=False,
        enable_asserts=False,
        num_devices=NCORES,
    )

    xT = nc.dram_tensor("xT", [128, DO, TT], BF16, kind="ExternalInput").ap()
    wq = nc.dram_tensor("wq", [128, DO, 128], BF16, kind="ExternalInput").ap()
    wk = nc.dram_tensor("wk", [128, DO, 128], BF16, kind="ExternalInput").ap()
    wv = nc.dram_tensor("wv", [128, DO, 128], BF16, kind="ExternalInput").ap()
    wo = nc.dram_tensor("wo", [128, DO, 1024], BF16, kind="ExternalInput").ap()
    bqin = nc.dram_tensor("bq", [128, 1], F32, kind="ExternalInput").ap()
    bkin = nc.dram_tensor("bk", [128, 1], F32, kind="ExternalInput").ap()
    bvin = nc.dram_tensor("bv", [128, 1], F32, kind="ExternalInput").ap()
    boin = nc.dram_tensor("bo", [1, 1024], F32, kind="ExternalInput").ap()
    tri_in = nc.dram_tensor("tri", [128, 128], BF16, kind="ExternalInput").ap()
    id_in = nc.dram_tensor("identf", [128, 128], F32, kind="ExternalInput").ap()
    sel_in = nc.dram_tensor("sel", [16, NCORES, 128], BF16, kind="ExternalInput").ap()
    if mode == "generic":
        mtiles = nc.dram_tensor(
            "mtiles", [n_mtiles, 128, 512], BF16, kind="ExternalInput"
        ).ap()
    y = nc.dram_tensor("y", [sum(GTOKS), 1024], F32, kind="ExternalOutput").ap()

    with tile.TileContext(nc) as tc, ExitStack() as ctx:
        pers = ctx.enter_context(tc.tile_pool(name="pers", bufs=1))
        # one PSUM pool; 8 banks total:
        #   tag A x2 bufs, 2 banks each = 4 (proj psums, transposes, attn ST,
        #                                    rb broadcasts, yproj)
        #   tag pv x2 bufs, 2 banks each = 4 (PV accumulators, double-buffered
        #                                     so the next pair starts while the
        #                                     previous drains to SBUF)
        pp = ctx.enter_context(tc.tile_pool(name="pp", bufs=2, space="PSUM"))
        dramp = ctx.enter_context(tc.tile_pool(name="dramp", bufs=1, space="DRAM"))

        # ---- persistent SBUF ----
        wq_sb = pers.tile([128, DO, 128], BF16, name="wq_sb")
        nc.sync.dma_start(wq_sb[:], wq[:])
        wk_sb = pers.tile([128, DO, 128], BF16, name="wk_sb")
        nc.sync.dma_start(wk_sb[:], wk[:])
        wv_sb = pers.tile([128, DO, 128], BF16, name="wv_sb")
        nc.sync.dma_start(wv_sb[:], wv[:])
        bq_sb = pers.tile([128, 1], F32, name="bq_sb")
        nc.sync.dma_start(bq_sb[:], bqin[:])
        bk_sb = pers.tile([128, 1], F32, name="bk_sb")
        nc.sync.dma_start(bk_sb[:], bkin[:])
        bv_sb = pers.tile([128, 1], F32, name="bv_sb")
        nc.sync.dma_start(bv_sb[:], bvin[:])
        tri_full = pers.tile([128, 128], BF16, name="tri_full")
        nc.sync.dma_start(tri_full[:], tri_in[:])
        tri_sb = tri_full[:, 0:128]
        ident_t = pers.tile([128, 128], F32, name="ident_t")
        nc.sync.dma_start(ident_t[:], id_in[:])
        ident = ident_t[:]
        sel_sb = pers.tile([16, NCORES, 128], BF16, name="sel_sb")
        nc.sync.dma_start(sel_sb[:], sel_in[:])

        ones_bf = pers.tile([128, 128], BF16, name="ones_bf")
        nc.vector.memset(ones_bf[:], 1.0)
        ones_f32 = pers.tile([1, 128], F32, name="ones_f32")
        nc.vector.memset(ones_f32[:], 1.0)

        qt = pers.tile([128, TT], BF16, name="qt")
        kt = pers.tile([128, TT], BF16, name="kt")
        vsb = pers.tile([128, B * NJ, HL, 80], BF16, name="vsb")
        nc.vector.tensor_copy(
            vsb[:, :, :, 64],
            ones_bf[:, 0 : B * NJ * HL].rearrange("p (a b) -> p a b", a=B * NJ),
        )
        # deferred persistents (needed only post-collective)
        wo_sb = pers.tile([128, DO, 1024], BF16, name="wo_sb")
        bo_sb = pers.tile([1, 1024], F32, name="bo_sb")
        bob = pers.tile([128, 1024], F32, name="bob")

        # a2a buffers per group: [chunk(=dest core), 130 rows, GTOK cols]
        # rows 0:65 = head0 (64 PV rows + den), 65:130 = head1
        a2a_in = [
            dramp.tile([NCORES, 2 * 65, GTOKS[g]], BF16, name=f"a2a_in{g}")
            for g in range(len(GROUPS))
        ]
        a2a_out = [
            dramp.tile([NCORES, 2 * 65, GTOKS[g]], BF16, name=f"a2a_out{g}")
            for g in range(len(GROUPS))
        ]

        xtp = ctx.enter_context(tc.tile_pool(name="xtp", bufs=2))
        vtp = ctx.enter_context(tc.tile_pool(name="vtp", bufs=2))
        sxp = ctx.enter_context(tc.tile_pool(name="sxp", bufs=3))
        otp_pool = ctx.enter_context(tc.tile_pool(name="otp", bufs=3))
        mtp = ctx.enter_context(tc.tile_pool(name="mtp", bufs=2))
        arp = ctx.enter_context(tc.tile_pool(name="arp", bufs=2))
        nap = ctx.enter_context(tc.tile_pool(name="nap", bufs=8))
        yp = ctx.enter_context(tc.tile_pool(name="yp", bufs=2))

        def emit_proj(tbp):
            """Projections for 1024 tokens starting at 1024*tbp."""
            t0 = 1024 * tbp
            xt_t = xtp.tile([128, DO, 1024], BF16, name=f"xt{tbp}", tag="xt")
            nc.sync.dma_start(xt_t[:], xT[:, :, t0 : t0 + 1024])
            for w_sb, b_sb, dst, nm in (
                (wq_sb, bq_sb, qt, "q"),
                (wk_sb, bk_sb, kt, "k"),
            ):
                ps_t = pp.tile([128, 1024], F32, name=f"ps{nm}{tbp}", tag="A")
                ps = ps_t[:]
                for do in range(DO):
                    for half in range(2):
                        nc.tensor.matmul(
                            ps[:, 512 * half : 512 * (half + 1)],
                            w_sb[:, do, :],
                            xt_t[:, do, 512 * half : 512 * (half + 1)],
                            start=(do == 0),
                            stop=(do == DO - 1),
                        )
                nc.vector.tensor_scalar_add(dst[:, t0 : t0 + 1024], ps, b_sb[:])
            vts = []
            for half in range(2):
                vps_t = pp.tile([128, 512], F32, name=f"vps{tbp}_{half}", tag="A")
                vps = vps_t[:]
                for do in range(DO):
                    nc.tensor.matmul(
                        vps,
                        wv_sb[:, do, :],
                        xt_t[:, do, 512 * half : 512 * (half + 1)],
                        start=(do == 0),
                        stop=(do == DO - 1),
                    )
                vt_t = vtp.tile([128, 512], F32, name=f"vt{tbp}_{half}", tag="vt")
                nc.vector.tensor_scalar_add(vt_t[:], vps, bv_sb[:])
                vts.append(vt_t)
            # transposes batched after the bf16 matmuls (fp32 transposes can
            # disable FWL for subsequent weight loads)
            for half in range(2):
                for k in range(4):
                    g = 8 * tbp + 4 * half + k  # global 128-token chunk
                    tps_t = pp.tile([128, 128], F32, name=f"tps{g}", tag="A")
                    tps = tps_t[:]
                    nc.tensor.transpose(
                        tps, vts[half][:, 128 * k : 128 * (k + 1)], ident
                    )
                    nc.vector.tensor_copy(
                        vsb[:, g, :, 0:64],
                        tps.rearrange("t (h c) -> t h c", h=HL),
                    )

        def emit_pair(g, k, b, a):
            """Attention for query chunk a of batch b; stage into group g."""
            ii0 = b * T + 512 * a
            if mode == "causal":
                jcs = list(range(4 * a + 4))
            elif mode == "ones":
                jcs = list(range(NJ))
            else:
                jcs = [jc for jc in range(NJ) if blocks[jc][a] != 0]
            otp = otp_pool.tile(
                [65, HL, 512], BF16, name=f"otp{b}_{a}", tag="ot"
            )
            if not jcs:
                nc.vector.memset(otp[:], 0.0)
                nc.vector.memset(otp[64:65, :, :], 1.0)
            else:
                pv_t = pp.tile(
                    [65, HL, 512], F32, name=f"pv{b}_{a}", tag="pv", bufs=2
                )
                pvs = [pv_t[:, h, :] for h in range(HL)]

                def emit_pv(pend):
                    ex, s, w, jc, first, last = pend
                    for h in range(HL):
                        nc.tensor.matmul(
                            pvs[h][:, s:512],
                            vsb[:, b * NJ + jc, h, 0:65],
                            ex[:, h, 0:w],
                            start=first,
                            stop=last,
                        )

                pend = None
                for idx, jc in enumerate(jcs):
                    j0 = b * T + 128 * jc
                    diag = mode == "causal" and jc >= 4 * a
                    s = 128 * (jc - 4 * a) if diag else 0
                    w = 512 - s
                    st = pp.tile(
                        [128, HL, 512], F32, name=f"st{b}_{a}_{jc}", tag="A"
                    )
                    for h in range(HL):
                        nc.tensor.matmul(
                            st[:, h, 0:w],
                            kt[64 * h : 64 * (h + 1), j0 : j0 + 128],
                            qt[64 * h : 64 * (h + 1), ii0 + s : ii0 + 512],
                            start=True,
                            stop=True,
                            tile_position=(64 * h, 0),
                        )
                    ex = sxp.tile(
                        [128, HL, 512], BF16, name=f"ex{b}_{a}_{jc}", tag="ex"
                    )
                    nc.scalar.activation(ex[:, :, 0:w], st[:, :, 0:w], AF.Exp)
                    if diag:
                        for h in range(HL):
                            nc.vector.tensor_mul(
                                ex[:, h, 0:128], ex[:, h, 0:128], tri_sb
                            )
                    if mode == "generic" and blocks[jc][a] != 1:
                        mt = mtp.tile(
                            [128, 512], BF16, name=f"mt{b}_{a}_{jc}", tag="mt"
                        )
                        nc.sync.dma_start(mt[:], mtiles[blocks[jc][a][1]])
                        for h in range(HL):
                            nc.vector.tensor_mul(ex[:, h, :], ex[:, h, :], mt[:])
                    if pend is not None:
                        emit_pv(pend)
                    pend = (ex, s, w, jc, idx == 0, idx == len(jcs) - 1)
                emit_pv(pend)
                # raw PV + denominator rows -> bf16 staging tile
                nc.vector.tensor_copy(otp[:], pv_t[:])
            # stage into a2a input (gpsimd queue: keeps attention-gated DMAs
            # off the sync queue); pair k covers chunks cpp*k .. cpp*(k+1)-1
            csz = GTOKS[g]
            cpp = 512 // csz
            for h in range(HL):
                for piece in range(cpp):
                    nc.gpsimd.dma_start(
                        a2a_in[g][cpp * k + piece, 65 * h : 65 * (h + 1), :],
                        otp[0:65, h, csz * piece : csz * (piece + 1)],
                    )

        def emit_collective(g):
            nc.gpsimd.collective_compute(
                "AllToAll",
                mybir.AluOpType.bypass,
                replica_groups=[list(range(NCORES))],
                ins=[a2a_in[g].opt()],
                outs=[a2a_out[g].opt()],
            )

        post_state = {}

        def emit_post_dma(g):
            """Receiver-side loads for group g (gpsimd queue, gated on cc g)."""
            csz = GTOKS[g]
            ag = a2a_out[g].rearrange("s (h r) c -> s h r c", h=2)
            araw = []
            for src in range(NCORES):
                t = arp.tile(
                    [128, csz], BF16, name=f"araw{g}_{src}", tag="araw", bufs=8
                )
                nc.gpsimd.dma_start(t[0:64, :], ag[src, 0, 0:64, :])
                nc.gpsimd.dma_start(t[64:128, :], ag[src, 1, 0:64, :])
                araw.append(t)
            densb = arp.tile([16, csz], BF16, name=f"densb{g}", tag="densb")
            nc.gpsimd.dma_start(
                densb[:],
                a2a_out[g]
                .rearrange("s (h r) c -> (s h) r c", h=2)[:, 64, :],
            )
            post_state[g] = (araw, densb)

        def emit_post_compute(g):
            """Normalize + output projection for group g."""
            csz = GTOKS[g]
            araw, densb = post_state[g]
            densf = arp.tile([16, csz], F32, name=f"densf{g}", tag="densf")
            nc.vector.tensor_copy(densf[:], densb[:])
            rcpf = arp.tile([16, csz], F32, name=f"rcpf{g}", tag="rcpf")
            nc.vector.reciprocal_approx_fast(rcpf[:], densf[:])
            rcpb = arp.tile([16, csz], BF16, name=f"rcpb{g}", tag="rcpb")
            nc.vector.tensor_copy(rcpb[:], rcpf[:])
            nas = []
            for src in range(NCORES):
                rb_t = pp.tile([128, csz], F32, name=f"rb{g}_{src}", tag="A")
                rb = rb_t[:, 0:csz]
                nc.tensor.matmul(
                    rb, sel_sb[:, src, :], rcpb[:], start=True, stop=True
                )
                na = nap.tile(
                    [128, csz], BF16, name=f"na{g}_{src}", tag="na", bufs=8
                )
                nc.vector.tensor_mul(na[:], araw[src][:], rb)
                nas.append(na)
            for ti in range(csz // 128):
                yps_t = pp.tile([128, 1024], F32, name=f"yps{g}_{ti}", tag="A")
                yps = yps_t[:]
                for src in range(NCORES):
                    for half in range(2):
                        nc.tensor.matmul(
                            yps[:, 512 * half : 512 * (half + 1)],
                            nas[src][:, 128 * ti : 128 * (ti + 1)],
                            wo_sb[:, src, 512 * half : 512 * (half + 1)],
                            start=(src == 0),
                            stop=(src == NCORES - 1),
                        )
                y_t = yp.tile([128, 1024], F32, name=f"y{g}_{ti}", tag="y")
                nc.vector.tensor_add(y_t[:], yps, bob[:])
                nc.sync.dma_start(
                    y[YOFF[g] + 128 * ti : YOFF[g] + 128 * (ti + 1), :], y_t[:]
                )

        # ---- emission schedule ----
        for tbp in (0, 2, 1, 3):
            emit_proj(tbp)
        nc.sync.dma_start(wo_sb[:], wo[:])
        nc.sync.dma_start(bo_sb[:], boin[:])
        for i in range(2):
            bps_t = pp.tile([128, 1024], F32, name=f"bps{i}", tag="A")
            nc.tensor.matmul(
                bps_t[:, 512 * i : 512 * (i + 1)],
                ones_f32[:, :],
                bo_sb[:, 512 * i : 512 * (i + 1)],
                start=True,
                stop=True,
            )
            nc.vector.tensor_copy(
                bob[:, 512 * i : 512 * (i + 1)],
                bps_t[:, 512 * i : 512 * (i + 1)],
            )
        for k, (b, a) in enumerate(GROUPS[0]):
            emit_pair(0, k, b, a)
        emit_collective(0)
        emit_post_dma(0)
        for k, (b, a) in enumerate(GROUPS[1]):
            emit_pair(1, k, b, a)
        emit_collective(1)
        emit_post_compute(0)
        emit_post_dma(1)
        for k, (b, a) in enumerate(GROUPS[2]):
            emit_pair(2, k, b, a)
        emit_collective(2)
        emit_post_compute(1)
        emit_post_dma(2)
        emit_post_compute(2)

    nc.compile()
    return nc


def _detect_mode(mask):
    m2 = np.asarray(mask).reshape(T, T)
    if np.array_equal(m2, np.tril(np.ones((T, T), m2.dtype))):
        return "causal", None, None
    if np.all(m2 != 0):
        return "ones", None, None
    # generic: classify [jc, a] blocks of mask^T
    mT = (m2 != 0).T.astype(np.float32)  # [j, i]
    blocks = [[0] * NI for _ in range(NJ)]
    tiles = []
    seen = {}
    for jc in range(NJ):
        for a in range(NI):
            sub = mT[128 * jc : 128 * (jc + 1), 512 * a : 512 * (a + 1)]
            if not sub.any():
                blocks[jc][a] = 0
            elif sub.all():
                blocks[jc][a] = 1
            else:
                key = sub.tobytes()
                if key not in seen:
                    seen[key] = len(tiles)
                    tiles.append(sub.copy())
                blocks[jc][a] = (2, seen[key])
    mt = np.stack(tiles) if tiles else np.zeros((1, 128, 512), np.float32)
    return "generic", blocks, mt


def _bf16(a):
    import ml_dtypes

    return np.ascontiguousarray(a, dtype=np.float32).astype(ml_dtypes.bfloat16)


def _rearr_w(w):
    # [D, M] -> [128, DO, M] as (d_inner, d_outer, m), bf16
    m = w.shape[1]
    return _bf16(
        np.ascontiguousarray(w, dtype=np.float32)
        .reshape(DO, 128, m)
        .transpose(1, 0, 2)
    )


def kernel(x, mask, Wq, bq, Wk, bk, Wv, bv, Wo, bo, trace=False):
    from concourse import bass_utils

    x = np.asarray(x, dtype=np.float32)
    Wq = np.asarray(Wq, dtype=np.float32)
    Wk = np.asarray(Wk, dtype=np.float32)
    Wv = np.asarray(Wv, dtype=np.float32)
    Wo = np.asarray(Wo, dtype=np.float32)
    bq = np.asarray(bq, dtype=np.float32)
    bk = np.asarray(bk, dtype=np.float32)
    bv = np.asarray(bv, dtype=np.float32)
    bo = np.asarray(bo, dtype=np.float32)

    mode, blocks, mt = _detect_mode(mask)
    cache_key = (mode, None if blocks is None else str(blocks))
    if cache_key not in _cache:
        _cache[cache_key] = _build_module(
            mode, blocks, 1 if mt is None else mt.shape[0]
        )
    nc = _cache[cache_key]

    scale = 1.0 / math.sqrt(D_K)
    xT_arr = _bf16(x.reshape(TT, D_MODEL).T.reshape(DO, 128, TT).transpose(1, 0, 2))
    wo_arr = _rearr_w(Wo)
    bo_arr = np.ascontiguousarray(bo.reshape(1, 1024))
    tri_arr = _bf16(np.triu(np.ones((128, 128), np.float32)))
    id_arr = np.eye(128, dtype=np.float32)
    # sel[k, s, p] = 1 iff k == 2*s + p//64  (denominator-broadcast selector)
    kk = np.arange(16)[:, None, None]
    ss = np.arange(NCORES)[None, :, None]
    ppp = np.arange(128)[None, None, :]
    sel_arr = _bf16((kk == 2 * ss + ppp // 64).astype(np.float32))

    in_maps = []
    for c in range(NCORES):
        sl = slice(128 * c, 128 * (c + 1))
        m = {
            "xT": xT_arr,
            "wq": _rearr_w(Wq[:, sl] * scale),
            "wk": _rearr_w(Wk[:, sl]),
            "wv": _rearr_w(Wv[:, sl]),
            "wo": wo_arr,
            "bq": np.ascontiguousarray((bq[sl] * scale).reshape(128, 1)),
            "bk": np.ascontiguousarray(bk[sl].reshape(128, 1)),
            "bv": np.ascontiguousarray(bv[sl].reshape(128, 1)),
            "bo": bo_arr,
            "tri": tri_arr,
            "identf": id_arr,
            "sel": sel_arr,
        }
        if mode == "generic":
            m["mtiles"] = _bf16(mt)
        in_maps.append(m)

    if trace:
        trace = _install_ntff_hook()
    res = bass_utils.run_bass_kernel_spmd(
        nc, in_maps, core_ids=list(range(NCORES)), trace=trace
    )
    # reassemble: core c, group g holds csz tokens of pair GROUPS[g][c//cpp]
    # at in-pair offset csz*(c%cpp)
    out = np.empty((B, T, D_MODEL), dtype=np.float32)
    for c in range(NCORES):
        yc = res.results[c]["y"]
        for g, pairs in enumerate(GROUPS):
            csz = GTOKS[g]
            cpp = 512 // csz
            b, a = pairs[c // cpp]
            t0 = 512 * a + csz * (c % cpp)
            out[b, t0 : t0 + csz] = yc[YOFF[g] : YOFF[g] + csz]
    if trace:
        kernel._last_result = res
    return out


# revision 32
# speedup vs baseline: 1.2196x; 1.2196x over previous
"""Causal self-attention Trainium2 Bass kernel (v2).

Sharding: 8-way head tensor-parallelism (2 heads per core, full batch) for
QKV projections + attention; the attention output is re-sharded over tokens
with TWO AllToAlls (pairs a<2 fire at ~30% of attention and hide under the
rest; pairs a>=2 fire at the end), then each core output-projects 512 tokens.

Key differences vs v1:
  - softmax normalization happens AFTER the collective: each (b,a) pair ships
    raw PV rows plus the denominator row (65 rows/head) through the AllToAll;
    the receiver batches all 16 denominators into one [16,256]
    reciprocal_approx_fast and broadcasts via a tiny selector matmul.
  - attention inner loop is software-pipelined (PV of chunk i deferred until
    after S/exp of chunk i+1) so the in-order PE queue never waits on ACT.
  - projections use N=1024 moving operands, are interleaved with attention
    emission, and the Wo load is deferred out of the startup critical path.

Per-core layouts:
  xT     [128, 8, 4096] bf16   x^T arranged (d_inner, d_outer, b*t)
  wq/wk/wv [128, 8, 128] bf16  W[:, head-slice] as (d_inner, d_outer, out)
  wo     [128, 8, 1024] bf16   full Wo
  Q^T/K^T in SBUF [128 (2 heads x 64), 4096] bf16
  V in SBUF [128 (j in chunk), 32 (b*jc), 2 (head), 80 (V | ones | pad)]
  S^T = K^T.T @ Q^T tiles [j=128, i<=512], 2 heads packed in PE row-halves
  PV accumulates [65, 2, 512] (65th row = denominator via ones column of V)
"""

import math
import os

import numpy as np

os.environ.setdefault("JAX_COMPILATION_CACHE_DIR", "/tmp/jax_cache")

D_MODEL = 1024
NUM_HEADS = 16
D_K = 64
B = 2
T = 2048
TT = B * T          # 4096 flattened tokens
NCORES = 8
HL = NUM_HEADS // NCORES   # heads per core = 2
DO = D_MODEL // 128        # 8 contraction chunks
NI = T // 512              # 4 query chunks per batch
NJ = T // 128              # 16 key chunks per batch

# pair processing order (b, a) and the AllToAll groups
PAIR_ORDER = [(0, 0), (1, 0), (0, 1), (1, 1), (0, 2), (1, 2), (0, 3), (1, 3)]
GROUPS = [PAIR_ORDER[:4], PAIR_ORDER[4:6], PAIR_ORDER[6:]]
GTOKS = [64 * len(p) for p in GROUPS]          # tokens per core per a2a
YOFF = [sum(GTOKS[:g]) for g in range(len(GROUPS))]

_cache = {}


def _install_ntff_hook():
    """The agent image's antenv lacks axon_hooks; replicate what
    trn_agent_boot would register so trace=True can capture NTFFs."""
    import sys
    import types

    try:
        from antenv import axon_hooks  # noqa: F401
        return True
    except ImportError:
        pass
    try:
        import antenv
        from trn_agent_boot.trn_boot import _ntff_profile_via_ctypes

        mod = types.ModuleType("antenv.axon_hooks")
        holder = [None]
        mod.set_axon_ntff_profile_hook = lambda h: holder.__setitem__(0, h)
        mod.get_axon_ntff_profile_hook = lambda: holder[0]
        sys.modules["antenv.axon_hooks"] = mod
        antenv.axon_hooks = mod
        mod.set_axon_ntff_profile_hook(
            _ntff_profile_via_ctypes("/opt/axon/libaxon_pjrt.so")
        )
        return True
    except Exception:
        return False


def _build_module(mode, blocks=None, n_mtiles=1):
    """Build + compile the Bass module.

    mode: "causal" (tril mask), "ones" (no masking), "generic"
    blocks: for generic mode, blocks[jc][a] = 0 skip / 1 full / (2, idx) mixed
    """
    from contextlib import ExitStack

    import concourse.mybir as mybir
    import concourse.tile as tile
    from concourse import bacc

    F32 = mybir.dt.float32
    BF16 = mybir.dt.bfloat16
    AF = mybir.ActivationFunctionType

    nc = bacc.Bacc(
        "TRN2",
        target_bir_lowering=False,
        debug=False,
        enable_asserts=False,
        num_devices=NCORES,
    )

    xT = nc.dram_tensor("xT", [128, DO, TT], BF16, kind="ExternalInput").ap()
    wq = nc.dram_tensor("wq", [128, DO, 128], BF16, kind="ExternalInput").ap()
    wk = nc.dram_tensor("wk", [128, DO, 128], BF16, kind="ExternalInput").ap()
    wv = nc.dram_tensor("wv", [128, DO, 128], BF16, kind="ExternalInput").ap()
    wo = nc.dram_tensor("wo", [128, DO, 1024], BF16, kind="ExternalInput").ap()
    bqin = nc.dram_tensor("bq", [128, 1], F32, kind="ExternalInput").ap()
    bkin = nc.dram_tensor("bk", [128, 1], F32, kind="ExternalInput").ap()
    bvin = nc.dram_tensor("bv", [128, 1], F32, kind="ExternalInput").ap()
    boin = nc.dram_tensor("bo", [1, 1024], F32, kind="ExternalInput").ap()
    tri_in = nc.dram_tensor("tri", [128, 128], BF16, kind="ExternalInput").ap()
    id_in = nc.dram_tensor("identf", [128, 128], F32, kind="ExternalInput").ap()
    sel_in = nc.dram_tensor("sel", [16, NCORES, 128], BF16, kind="ExternalInput").ap()
    if mode == "generic":
        mtiles = nc.dram_tensor(
            "mtiles", [n_mtiles, 128, 512], BF16, kind="ExternalInput"
        ).ap()
    y = nc.dram_tensor("y", [sum(GTOKS), 1024], F32, kind="ExternalOutput").ap()

    with tile.TileContext(nc) as tc, ExitStack() as ctx:
        pers = ctx.enter_context(tc.tile_pool(name="pers", bufs=1))
        # one PSUM pool; 8 banks total:
        #   tag A x2 bufs, 2 banks each = 4 (proj psums, transposes, attn ST,
        #                                    rb broadcasts, yproj)
        #   tag pv x2 bufs, 2 banks each = 4 (PV accumulators, double-buffered
        #                                     so the next pair starts while the
        #                                     previous drains to SBUF)
        pp = ctx.enter_context(tc.tile_pool(name="pp", bufs=2, space="PSUM"))
        dramp = ctx.enter_context(tc.tile_pool(name="dramp", bufs=1, space="DRAM"))

        # ---- persistent SBUF ----
        wq_sb = pers.tile([128, DO, 128], BF16, name="wq_sb")
        nc.sync.dma_start(wq_sb[:], wq[:])
        wk_sb = pers.tile([128, DO, 128], BF16, name="wk_sb")
        nc.sync.dma_start(wk_sb[:], wk[:])
        wv_sb = pers.tile([128, DO, 128], BF16, name="wv_sb")
        nc.sync.dma_start(wv_sb[:], wv[:])
        bq_sb = pers.tile([128, 1], F32, name="bq_sb")
        nc.sync.dma_start(bq_sb[:], bqin[:])
        bk_sb = pers.tile([128, 1], F32, name="bk_sb")
        nc.sync.dma_start(bk_sb[:], bkin[:])
        bv_sb = pers.tile([128, 1], F32, name="bv_sb")
        nc.sync.dma_start(bv_sb[:], bvin[:])
        tri_full = pers.tile([128, 128], BF16, name="tri_full")
        nc.sync.dma_start(tri_full[:], tri_in[:])
        tri_sb = tri_full[:, 0:128]
        ident_t = pers.tile([128, 128], F32, name="ident_t")
        nc.sync.dma_start(ident_t[:], id_in[:])
        ident = ident_t[:]
        sel_sb = pers.tile([16, NCORES, 128], BF16, name="sel_sb")
        nc.sync.dma_start(sel_sb[:], sel_in[:])

        ones_bf = pers.tile([128, 128], BF16, name="ones_bf")
        nc.vector.memset(ones_bf[:], 1.0)
        ones_f32 = pers.tile([1, 128], F32, name="ones_f32")
        nc.vector.memset(ones_f32[:], 1.0)

        qt = pers.tile([128, TT], BF16, name="qt")
        kt = pers.tile([128, TT], BF16, name="kt")
        vsb = pers.tile([128, B * NJ, HL, 80], BF16, name="vsb")
        nc.vector.tensor_copy(
            vsb[:, :, :, 64],
            ones_bf[:, 0 : B * NJ * HL].rearrange("p (a b) -> p a b", a=B * NJ),
        )
        # deferred persistents (needed only post-collective)
        wo_sb = pers.tile([128, DO, 1024], BF16, name="wo_sb")
        bo_sb = pers.tile([1, 1024], F32, name="bo_sb")
        bob = pers.tile([128, 1024], F32, name="bob")

        # a2a buffers per group: [chunk(=dest core), 130 rows, GTOK cols]
        # rows 0:65 = head0 (64 PV rows + den), 65:130 = head1
        a2a_in = [
            dramp.tile([NCORES, 2 * 65, GTOKS[g]], BF16, name=f"a2a_in{g}")
            for g in range(len(GROUPS))
        ]
        a2a_out = [
            dramp.tile([NCORES, 2 * 65, GTOKS[g]], BF16, name=f"a2a_out{g}")
            for g in range(len(GROUPS))
        ]

        xtp = ctx.enter_context(tc.tile_pool(name="xtp", bufs=2))
        vtp = ctx.enter_context(tc.tile_pool(name="vtp", bufs=2))
        sxp = ctx.enter_context(tc.tile_pool(name="sxp", bufs=3))
        otp_pool = ctx.enter_context(tc.tile_pool(name="otp", bufs=3))
        mtp = ctx.enter_context(tc.tile_pool(name="mtp", bufs=2))
        arp = ctx.enter_context(tc.tile_pool(name="arp", bufs=2))
        nap = ctx.enter_context(tc.tile_pool(name="nap", bufs=8))
        yp = ctx.enter_context(tc.tile_pool(name="yp", bufs=2))

        def emit_proj(tbp):
            """Projections for 1024 tokens starting at 1024*tbp."""
            t0 = 1024 * tbp
            xt_t = xtp.tile([128, DO, 1024], BF16, name=f"xt{tbp}", tag="xt")
            nc.sync.dma_start(xt_t[:], xT[:, :, t0 : t0 + 1024])
            for w_sb, b_sb, dst, nm in (
                (wq_sb, bq_sb, qt, "q"),
                (wk_sb, bk_sb, kt, "k"),
            ):
                ps_t = pp.tile([128, 1024], F32, name=f"ps{nm}{tbp}", tag="A")
                ps = ps_t[:]
                for do in range(DO):
                    for half in range(2):
                        nc.tensor.matmul(
                            ps[:, 512 * half : 512 * (half + 1)],
                            w_sb[:, do, :],
                            xt_t[:, do, 512 * half : 512 * (half + 1)],
                            start=(do == 0),
                            stop=(do == DO - 1),
                        )
                nc.vector.tensor_scalar_add(dst[:, t0 : t0 + 1024], ps, b_sb[:])
            vts = []
            for half in range(2):
                vps_t = pp.tile([128, 512], F32, name=f"vps{tbp}_{half}", tag="pv")
                vps = vps_t[:]
                for do in range(DO):
                    nc.tensor.matmul(
                        vps,
                        wv_sb[:, do, :],
                        xt_t[:, do, 512 * half : 512 * (half + 1)],
                        start=(do == 0),
                        stop=(do == DO - 1),
                    )
                vt_t = vtp.tile([128, 512], F32, name=f"vt{tbp}_{half}", tag="vt")
                nc.vector.tensor_scalar_add(vt_t[:], vps, bv_sb[:])
                vts.append(vt_t)
            # transposes batched after the bf16 matmuls (fp32 transposes can
            # disable FWL for subsequent weight loads)
            for half in range(2):
                for k in range(4):
                    g = 8 * tbp + 4 * half + k  # global 128-token chunk
                    tps_t = pp.tile([128, 128], F32, name=f"tps{g}", tag="pv")
                    tps = tps_t[:]
                    nc.tensor.transpose(
                        tps, vts[half][:, 128 * k : 128 * (k + 1)], ident
                    )
                    nc.vector.tensor_copy(
                        vsb[:, g, :, 0:64],
                        tps.rearrange("t (h c) -> t h c", h=HL),
                    )

        def emit_pair(g, k, b, a):
            """Attention for query chunk a of batch b; stage into group g."""
            ii0 = b * T + 512 * a
            if mode == "causal":
                jcs = list(range(4 * a + 4))
            elif mode == "ones":
                jcs = list(range(NJ))
            else:
                jcs = [jc for jc in range(NJ) if blocks[jc][a] != 0]
            otp = otp_pool.tile(
                [65, HL, 512], BF16, name=f"otp{b}_{a}", tag="ot"
            )
            if not jcs:
                nc.vector.memset(otp[:], 0.0)
                nc.vector.memset(otp[64:65, :, :], 1.0)
            else:
                pv_t = pp.tile(
                    [65, HL, 512], F32, name=f"pv{b}_{a}", tag="pv", bufs=2
                )
                pvs = [pv_t[:, h, :] for h in range(HL)]

                def emit_pv(pend):
                    ex, s, w, jc, first, last = pend
                    for h in range(HL):
                        nc.tensor.matmul(
                            pvs[h][:, s:512],
                            vsb[:, b * NJ + jc, h, 0:65],
                            ex[:, h, 0:w],
                            start=first,
                            stop=last,
                        )

                pend = None
                for idx, jc in enumerate(jcs):
                    j0 = b * T + 128 * jc
                    diag = mode == "causal" and jc >= 4 * a
                    s = 128 * (jc - 4 * a) if diag else 0
                    w = 512 - s
                    st = pp.tile(
                        [128, HL, 512], F32, name=f"st{b}_{a}_{jc}", tag="A"
                    )
                    for h in range(HL):
                        nc.tensor.matmul(
                            st[:, h, 0:w],
                            kt[64 * h : 64 * (h + 1), j0 : j0 + 128],
                            qt[64 * h : 64 * (h + 1), ii0 + s : ii0 + 512],
                            start=True,
                            stop=True,
                            tile_position=(64 * h, 0),
                        )
                    ex = sxp.tile(
                        [128, HL, 512], BF16, name=f"ex{b}_{a}_{jc}", tag="ex"
                    )
                    nc.scalar.activation(ex[:, :, 0:w], st[:, :, 0:w], AF.Exp)
                    if diag:
                        for h in range(HL):
                            nc.vector.tensor_mul(
                                ex[:, h, 0:128], ex[:, h, 0:128], tri_sb
                            )
                    if mode == "generic" and blocks[jc][a] != 1:
                        mt = mtp.tile(
                            [128, 512], BF16, name=f"mt{b}_{a}_{jc}", tag="mt"
                        )
                        nc.sync.dma_start(mt[:], mtiles[blocks[jc][a][1]])
                        for h in range(HL):
                            nc.vector.tensor_mul(ex[:, h, :], ex[:, h, :], mt[:])
                    if pend is not None:
                        emit_pv(pend)
                    pend = (ex, s, w, jc, idx == 0, idx == len(jcs) - 1)
                emit_pv(pend)
                # raw PV + denominator rows -> bf16 staging tile
                nc.vector.tensor_copy(otp[:], pv_t[:])
            # stage into a2a input (gpsimd queue: keeps attention-gated DMAs
            # off the sync queue); pair k covers chunks cpp*k .. cpp*(k+1)-1;
            # one DMA per head with the piece-split on the DRAM side
            csz = GTOKS[g]
            cpp = 512 // csz
            for h in range(HL):
                nc.gpsimd.dma_start(
                    a2a_in[g][cpp * k : cpp * (k + 1), 65 * h : 65 * (h + 1), :]
                    .rearrange("piece p c -> p piece c"),
                    otp[0:65, h, :].rearrange("p (piece c) -> p piece c", piece=cpp),
                )

        def emit_collective(g):
            nc.gpsimd.collective_compute(
                "AllToAll",
                mybir.AluOpType.bypass,
                replica_groups=[list(range(NCORES))],
                ins=[a2a_in[g].opt()],
                outs=[a2a_out[g].opt()],
            )

        post_state = {}

        def emit_post_dma(g):
            """Receiver-side loads for group g (gpsimd queue, gated on cc g).

            araw[p, src, c]: p = local dim within src's 128-dim block
            (rows 0:64 = head0 of src, 64:128 = head1)."""
            csz = GTOKS[g]
            ag = a2a_out[g].rearrange("s (h r) c -> s h r c", h=2)
            araw = arp.tile(
                [128, NCORES, csz], BF16, name=f"araw{g}", tag="araw", bufs=2
            )
            nc.gpsimd.dma_start(
                araw[0:64, :, :], ag[:, 0, 0:64, :].rearrange("s r c -> r s c")
            )
            nc.gpsimd.dma_start(
                araw[64:128, :, :], ag[:, 1, 0:64, :].rearrange("s r c -> r s c")
            )
            densb = arp.tile([16, csz], BF16, name=f"densb{g}", tag="densb")
            nc.gpsimd.dma_start(
                densb[:],
                a2a_out[g]
                .rearrange("s (h r) c -> (s h) r c", h=2)[:, 64, :],
            )
            post_state[g] = (araw, densb)

        def emit_post_compute(g):
            """Normalize + output projection for group g."""
            csz = GTOKS[g]
            araw, densb = post_state[g]
            densf = arp.tile([16, csz], F32, name=f"densf{g}", tag="densf")
            nc.vector.tensor_copy(densf[:], densb[:])
            rcpf = arp.tile([16, csz], F32, name=f"rcpf{g}", tag="rcpf")
            nc.vector.reciprocal_approx_fast(rcpf[:], densf[:])
            rcpb = arp.tile([16, csz], BF16, name=f"rcpb{g}", tag="rcpb")
            nc.vector.tensor_copy(rcpb[:], rcpf[:])
            na = nap.tile(
                [128, NCORES, csz], BF16, name=f"na{g}", tag="na", bufs=2
            )
            for src in range(NCORES):
                rb_t = pp.tile([128, csz], F32, name=f"rb{g}_{src}", tag="A")
                rb = rb_t[:, 0:csz]
                nc.tensor.matmul(
                    rb, sel_sb[:, src, :], rcpb[:], start=True, stop=True
                )
                nc.vector.tensor_mul(na[:, src, :], araw[:, src, :], rb)
            for ti in range(csz // 128):
                yps_t = pp.tile([128, 1024], F32, name=f"yps{g}_{ti}", tag="A")
                yps = yps_t[:]
                for src in range(NCORES):
                    for half in range(2):
                        nc.tensor.matmul(
                            yps[:, 512 * half : 512 * (half + 1)],
                            na[:, src, 128 * ti : 128 * (ti + 1)],
                            wo_sb[:, src, 512 * half : 512 * (half + 1)],
                            start=(src == 0),
                            stop=(src == NCORES - 1),
                        )
                y_t = yp.tile([128, 1024], F32, name=f"y{g}_{ti}", tag="y")
                nc.vector.tensor_add(y_t[:], yps, bob[:])
                nc.sync.dma_start(
                    y[YOFF[g] + 128 * ti : YOFF[g] + 128 * (ti + 1), :], y_t[:]
                )

        # ---- emission schedule ----
        # tiny warm-up collective: absorbs the ~11us first-collective latency
        # while projections run
        warm_in = dramp.tile([NCORES, 2, 16], BF16, name="warm_in")
        warm_out = dramp.tile([NCORES, 2, 16], BF16, name="warm_out")
        nc.gpsimd.collective_compute(
            "AllToAll",
            mybir.AluOpType.bypass,
            replica_groups=[list(range(NCORES))],
            ins=[warm_in.opt()],
            outs=[warm_out.opt()],
        )
        for tbp in (0, 2, 1, 3):
            emit_proj(tbp)
        nc.sync.dma_start(wo_sb[:], wo[:])
        nc.sync.dma_start(bo_sb[:], boin[:])
        for i in range(2):
            bps_t = pp.tile([128, 1024], F32, name=f"bps{i}", tag="A")
            nc.tensor.matmul(
                bps_t[:, 512 * i : 512 * (i + 1)],
                ones_f32[:, :],
                bo_sb[:, 512 * i : 512 * (i + 1)],
                start=True,
                stop=True,
            )
            nc.vector.tensor_copy(
                bob[:, 512 * i : 512 * (i + 1)],
                bps_t[:, 512 * i : 512 * (i + 1)],
            )
        for k, (b, a) in enumerate(GROUPS[0]):
            emit_pair(0, k, b, a)
        emit_collective(0)
        for k, (b, a) in enumerate(GROUPS[1]):
            emit_pair(1, k, b, a)
        emit_collective(1)
        emit_post_dma(0)
        emit_pair(2, 0, *GROUPS[2][0])
        emit_post_dma(1)
        emit_pair(2, 1, *GROUPS[2][1])
        emit_collective(2)
        emit_post_compute(0)
        emit_post_compute(1)
        emit_post_dma(2)
        emit_post_compute(2)

    nc.compile()
    return nc


def _detect_mode(mask):
    m2 = np.asarray(mask).reshape(T, T)
    if np.array_equal(m2, np.tril(np.ones((T, T), m2.dtype))):
        return "causal", None, None
    if np.all(m2 != 0):
        return "ones", None, None
    # generic: classify [jc, a] blocks of mask^T
    mT = (m2 != 0).T.astype(np.float32)  # [j, i]
    blocks = [[0] * NI for _ in range(NJ)]
    tiles = []
    seen = {}
    for jc in range(NJ):
        for a in range(NI):
            sub = mT[128 * jc : 128 * (jc + 1), 512 * a : 512 * (a + 1)]
            if not sub.any():
                blocks[jc][a] = 0
            elif sub.all():
                blocks[jc][a] = 1
            else:
                key = sub.tobytes()
                if key not in seen:
                    seen[key] = len(tiles)
                    tiles.append(sub.copy())
                blocks[jc][a] = (2, seen[key])
    mt = np.stack(tiles) if tiles else np.zeros((1, 128, 512), np.float32)
    return "generic", blocks, mt


def _bf16(a):
    import ml_dtypes

    return np.ascontiguousarray(a, dtype=np.float32).astype(ml_dtypes.bfloat16)


def _rearr_w(w):
    # [D, M] -> [128, DO, M] as (d_inner, d_outer, m), bf16
    m = w.shape[1]
    return _bf16(
        np.ascontiguousarray(w, dtype=np.float32)
        .reshape(DO, 128, m)
        .transpose(1, 0, 2)
    )


def kernel(x, mask, Wq, bq, Wk, bk, Wv, bv, Wo, bo, trace=False):
    from concourse import bass_utils

    x = np.asarray(x, dtype=np.float32)
    Wq = np.asarray(Wq, dtype=np.float32)
    Wk = np.asarray(Wk, dtype=np.float32)
    Wv = np.asarray(Wv, dtype=np.float32)
    Wo = np.asarray(Wo, dtype=np.float32)
    bq = np.asarray(bq, dtype=np.float32)
    bk = np.asarray(bk, dtype=np.float32)
    bv = np.asarray(bv, dtype=np.float32)
    bo = np.asarray(bo, dtype=np.float32)

    mode, blocks, mt = _detect_mode(mask)
    cache_key = (mode, None if blocks is None else str(blocks))
    if cache_key not in _cache:
        _cache[cache_key] = _build_module(
            mode, blocks, 1 if mt is None else mt.shape[0]
        )
    nc = _cache[cache_key]

    scale = 1.0 / math.sqrt(D_K)
    xT_arr = _bf16(x.reshape(TT, D_MODEL).T.reshape(DO, 128, TT).transpose(1, 0, 2))
    wo_arr = _rearr_w(Wo)
    bo_arr = np.ascontiguousarray(bo.reshape(1, 1024))
    tri_arr = _bf16(np.triu(np.ones((128, 128), np.float32)))
    id_arr = np.eye(128, dtype=np.float32)
    # sel[k, s, p] = 1 iff k == 2*s + p//64  (denominator-broadcast selector)
    kk = np.arange(16)[:, None, None]
    ss = np.arange(NCORES)[None, :, None]
    ppp = np.arange(128)[None, None, :]
    sel_arr = _bf16((kk == 2 * ss + ppp // 64).astype(np.float32))

    in_maps = []
    for c in range(NCORES):
        sl = slice(128 * c, 128 * (c + 1))
        m = {
            "xT": xT_arr,
            "wq": _rearr_w(Wq[:, sl] * scale),
            "wk": _rearr_w(Wk[:, sl]),
            "wv": _rearr_w(Wv[:, sl]),
            "wo": wo_arr,
            "bq": np.ascontiguousarray((bq[sl] * scale).reshape(128, 1)),
            "bk": np.ascontiguousarray(bk[sl].reshape(128, 1)),
            "bv": np.ascontiguousarray(bv[sl].reshape(128, 1)),
            "bo": bo_arr,
            "tri": tri_arr,
            "identf": id_arr,
            "sel": sel_arr,
        }
        if mode == "generic":
            m["mtiles"] = _bf16(mt)
        in_maps.append(m)

    if trace:
        trace = _install_ntff_hook()
    res = bass_utils.run_bass_kernel_spmd(
        nc, in_maps, core_ids=list(range(NCORES)), trace=trace
    )
    # reassemble: core c, group g holds csz tokens of pair GROUPS[g][c//cpp]
    # at in-pair offset csz*(c%cpp)
    out = np.empty((B, T, D_MODEL), dtype=np.float32)
    for c in range(NCORES):
        yc = res.results[c]["y"]
        for g, pairs in enumerate(GROUPS):
            csz = GTOKS[g]
            cpp = 512 // csz
            b, a = pairs[c // cpp]
            t0 = 512 * a + csz * (c % cpp)
            out[b, t0 : t0 + csz] = yc[YOFF[g] : YOFF[g] + csz]
    if trace:
        kernel._last_result = res
    return out
